# revision 11
# baseline (speedup 1.0000x reference)
"""SwiGLU FFN (dense MoE expert) on 8 TRN2 NeuronCores, tensor-parallel.

Reference computation (fp32):
    out = (silu(x @ w1.T) * (x @ w3.T)) @ w2.T
    x  [T=4096, H=4096]
    w1 [I=14336, H]  gate proj
    w3 [I=14336, H]  up proj
    w2 [H, I]        down proj

Sharding: tensor-parallel over the intermediate dim I. Each of the 8
cores owns I/8 = 1792 rows of w1/w3 and the matching 1792 columns of
w2, computes a full-shape partial output, and the host sums the 8
partials (the "all-reduce" of the row-parallel down projection).

On-core dataflow (per token block TB):
    A:  gate^T[i,t] = sum_h w1[i,h] x[t,h]   (psum, bf16 matmuls)
    B:  up^T  [i,t] = sum_h w3[i,h] x[t,h]
        h^T = silu(gate^T) * up^T            (ACT + DVE, -> bf16)
    C:  out[t,hd]  += sum_i h^T[i,t] w2[hd,i]

All matmul operands are pre-laid-out on the host so every DMA is a
contiguous [128, big] block and every matmul is a full 128-contraction
with a 512-wide moving operand.
"""

import numpy as np
import ml_dtypes

import concourse.bass as bass
import concourse.mybir as mybir
import concourse.tile as tile
from concourse import bacc, bass_utils

BF16 = mybir.dt.bfloat16
FP32 = mybir.dt.float32
NP_BF16 = ml_dtypes.bfloat16

TOKENS = 4096
HIDDEN = 4096
INTER = 14336
NCORES = 8
ILOC = INTER // NCORES  # 1792 intermediate rows per core


def build_ffn_bass(T, H, iloc, TB, NG):
    """Per-core Bass program (identical on all cores; SPMD over shards).

    T tokens, H hidden, iloc local intermediate rows, TB token block
    (<=512, psum free dim), NG down-proj output group width (<=512).

    DRAM layouts (host-prepped, see kernel()):
      xt  [128, HK, T]        bf16  xt[p,k,t]   = x[t, 128k+p]
      w1p [IT, 128, HK, 128]  bf16  w1p[i,p,k,j]= w1loc[128i+j, 128k+p]
      w3p same as w1p
      w2p [G, 128, IT, NG]    bf16  w2p[g,p,c,j]= w2[g*NG+j, i0+128c+p]
      out [T, H] fp32 partial (summed across cores on the host)
    """
    HK = H // 128
    IT = iloc // 128
    NB = T // TB
    G = H // NG
    TS = TB // 128
    assert TB <= 512 and NG <= 512

    nc = bacc.Bacc("TRN2", num_devices=NCORES)
    xt_d = nc.dram_tensor("xt", [128, HK, T], BF16, kind="ExternalInput")
    w1_d = nc.dram_tensor("w1p", [IT, 128, HK, 128], BF16, kind="ExternalInput")
    w3_d = nc.dram_tensor("w3p", [IT, 128, HK, 128], BF16, kind="ExternalInput")
    w2_d = nc.dram_tensor("w2p", [G, 128, IT, NG], BF16, kind="ExternalInput")
    out_d = nc.dram_tensor("out", [T, H], FP32, kind="ExternalOutput")

    with tile.TileContext(nc) as tc:
        with (
            tc.tile_pool(name="xt", bufs=2) as xt_pool,
            tc.tile_pool(name="w13", bufs=3) as w13_pool,
            tc.tile_pool(name="w2", bufs=2) as w2_pool,
            tc.tile_pool(name="h", bufs=2 * IT) as h_pool,
            tc.tile_pool(name="act", bufs=4) as act_pool,
            tc.tile_pool(name="ob", bufs=4) as ob_pool,
            tc.tile_pool(name="pg", bufs=2, space="PSUM") as pg_pool,
            tc.tile_pool(name="pu", bufs=2, space="PSUM") as pu_pool,
            tc.tile_pool(name="po", bufs=2, space="PSUM") as po_pool,
        ):
            XSPLIT = 4 if HK % 4 == 0 else 1
            for b in range(NB):
                t0 = b * TB
                # xt on the gpsimd SWDGE queue, split into XSPLIT pieces:
                # decouples from the weight-panel HWDGE ring so the next
                # block's activations land before the PE drains this block.
                xt_sb = xt_pool.tile([128, HK, TB], BF16)
                kstep = HK // XSPLIT
                for xs in range(XSPLIT):
                    k0 = xs * kstep
                    nc.gpsimd.dma_start(
                        xt_sb[:, k0 : k0 + kstep, :],
                        xt_d[:, k0 : k0 + kstep, t0 : t0 + TB],
                    )
                h_tiles = []
                for i in range(IT):
                    w1_sb = w13_pool.tile([128, HK, 128], BF16, tag="w1")
                    w3_sb = w13_pool.tile([128, HK, 128], BF16, tag="w3")
                    for xs in range(XSPLIT):
                        k0 = xs * kstep
                        nc.sync.dma_start(
                            w1_sb[:, k0 : k0 + kstep, :],
                            w1_d[i, :, k0 : k0 + kstep, :],
                        )
                        nc.sync.dma_start(
                            w3_sb[:, k0 : k0 + kstep, :],
                            w3_d[i, :, k0 : k0 + kstep, :],
                        )
                    pg = pg_pool.tile([128, TB], FP32)
                    pu = pu_pool.tile([128, TB], FP32)
                    for k in range(HK):
                        nc.tensor.matmul(
                            pg[:], w1_sb[:, k, :], xt_sb[:, k, :],
                            start=(k == 0), stop=(k == HK - 1),
                        )
                    for k in range(HK):
                        nc.tensor.matmul(
                            pu[:], w3_sb[:, k, :], xt_sb[:, k, :],
                            start=(k == 0), stop=(k == HK - 1),
                        )
                    sig = act_pool.tile([128, TB], FP32, tag="sig")
                    nc.scalar.activation(
                        sig[:], pg[:], mybir.ActivationFunctionType.Sigmoid
                    )
                    t1 = act_pool.tile([128, TB], FP32, tag="t1")
                    nc.vector.tensor_mul(t1[:], sig[:], pg[:])
                    h = h_pool.tile([128, TB], BF16)
                    nc.vector.tensor_mul(h[:], t1[:], pu[:])
                    h_tiles.append(h)
                for g in range(G):
                    w2_sb = w2_pool.tile([128, IT, NG], BF16)
                    nc.scalar.dma_start(w2_sb[:], w2_d[g])
                    for s in range(TS):
                        po = po_pool.tile([128, NG], FP32)
                        for c in range(IT):
                            nc.tensor.matmul(
                                po[:],
                                h_tiles[c][:, s * 128 : (s + 1) * 128],
                                w2_sb[:, c, :],
                                start=(c == 0), stop=(c == IT - 1),
                            )
                        ob = ob_pool.tile([128, NG], FP32)
                        nc.vector.tensor_copy(ob[:], po[:])
                        nc.scalar.dma_start(
                            out_d[t0 + s * 128 : t0 + (s + 1) * 128,
                                  g * NG : (g + 1) * NG],
                            ob[:],
                        )
    nc.compile()
    return nc


def prep_inputs(x, w1, w2, w3, ncores, TB, NG):
    """Host-side shard + layout prep. Returns per-core input maps."""
    T, H = x.shape
    I = w1.shape[0]
    iloc = I // ncores
    HK = H // 128
    IT = iloc // 128
    G = H // NG

    xt = np.ascontiguousarray(
        x.reshape(T, HK, 128).transpose(2, 1, 0)
    ).astype(NP_BF16)

    in_maps = []
    for c in range(ncores):
        i0 = c * iloc
        w1p = (
            w1[i0 : i0 + iloc]
            .reshape(IT, 128, HK, 128)
            .transpose(0, 3, 2, 1)
            .astype(NP_BF16)
        )
        w3p = (
            w3[i0 : i0 + iloc]
            .reshape(IT, 128, HK, 128)
            .transpose(0, 3, 2, 1)
            .astype(NP_BF16)
        )
        w2p = (
            w2[:, i0 : i0 + iloc]
            .reshape(G, NG, IT, 128)
            .transpose(0, 3, 2, 1)
            .astype(NP_BF16)
        )
        in_maps.append(
            {
                "xt": xt,
                "w1p": np.ascontiguousarray(w1p),
                "w3p": np.ascontiguousarray(w3p),
                "w2p": np.ascontiguousarray(w2p),
            }
        )
    return in_maps


_NC_CACHE = {}


def _get_nc(T, H, iloc, TB, NG):
    key = (T, H, iloc, TB, NG)
    if key not in _NC_CACHE:
        _NC_CACHE[key] = build_ffn_bass(T, H, iloc, TB, NG)
    return _NC_CACHE[key]


def run(x, w1, w2, w3, ncores=NCORES, TB=512, NG=512, **spmd_kwargs):
    """Full pipeline; returns (output, BassKernelResults)."""
    T, H = x.shape
    iloc = w1.shape[0] // ncores
    nc = _get_nc(T, H, iloc, TB, NG)
    in_maps = prep_inputs(x, w1, w2, w3, ncores, TB, NG)
    res = bass_utils.run_bass_kernel_spmd(
        nc, in_maps, list(range(ncores)), **spmd_kwargs
    )
    out = res.results[0]["out"]
    for c in range(1, ncores):
        out = out + res.results[c]["out"]
    return np.ascontiguousarray(out, dtype=np.float32), res


def kernel(x, w1, w2, w3):
    out, _ = run(
        np.asarray(x, dtype=np.float32),
        np.asarray(w1, dtype=np.float32),
        np.asarray(w2, dtype=np.float32),
        np.asarray(w3, dtype=np.float32),
    )
    return out


# revision 12
# speedup vs baseline: 1.0023x; 1.0023x over previous
"""SwiGLU FFN (dense MoE expert) on 8 TRN2 NeuronCores, tensor-parallel.

Reference computation (fp32):
    out = (silu(x @ w1.T) * (x @ w3.T)) @ w2.T
    x  [T=4096, H=4096]
    w1 [I=14336, H]  gate proj
    w3 [I=14336, H]  up proj
    w2 [H, I]        down proj

Sharding: tensor-parallel over the intermediate dim I. Each of the 8
cores owns I/8 = 1792 rows of w1/w3 and the matching 1792 columns of
w2, computes a full-shape partial output, and the host sums the 8
partials (the "all-reduce" of the row-parallel down projection).

On-core dataflow (per token block TB):
    A:  gate^T[i,t] = sum_h w1[i,h] x[t,h]   (psum, bf16 matmuls)
    B:  up^T  [i,t] = sum_h w3[i,h] x[t,h]
        h^T = silu(gate^T) * up^T            (ACT + DVE, -> bf16)
    C:  out[t,hd]  += sum_i h^T[i,t] w2[hd,i]

All matmul operands are pre-laid-out on the host so every DMA is a
contiguous [128, big] block and every matmul is a full 128-contraction
with a 512-wide moving operand.
"""

import numpy as np
import ml_dtypes

import concourse.bass as bass
import concourse.mybir as mybir
import concourse.tile as tile
from concourse import bacc, bass_utils

BF16 = mybir.dt.bfloat16
FP32 = mybir.dt.float32
NP_BF16 = ml_dtypes.bfloat16

TOKENS = 4096
HIDDEN = 4096
INTER = 14336
NCORES = 8
ILOC = INTER // NCORES  # 1792 intermediate rows per core


def build_ffn_bass(T, H, iloc, TB, NG):
    """Per-core Bass program (identical on all cores; SPMD over shards).

    T tokens, H hidden, iloc local intermediate rows, TB token block
    (<=512, psum free dim), NG down-proj output group width (<=512).

    DRAM layouts (host-prepped, see kernel()):
      xt  [128, HK, T]        bf16  xt[p,k,t]   = x[t, 128k+p]
      w1p [IT, 128, HK, 128]  bf16  w1p[i,p,k,j]= w1loc[128i+j, 128k+p]
      w3p same as w1p
      w2p [G, 128, IT, NG]    bf16  w2p[g,p,c,j]= w2[g*NG+j, i0+128c+p]
      out [T, H] fp32 partial (summed across cores on the host)
    """
    HK = H // 128
    IT = iloc // 128
    NB = T // TB
    G = H // NG
    TS = TB // 128
    assert TB <= 512 and NG <= 512

    nc = bacc.Bacc("TRN2", num_devices=NCORES)
    xt_d = nc.dram_tensor("xt", [128, HK, T], BF16, kind="ExternalInput")
    w1_d = nc.dram_tensor("w1p", [IT, 128, HK, 128], BF16, kind="ExternalInput")
    w3_d = nc.dram_tensor("w3p", [IT, 128, HK, 128], BF16, kind="ExternalInput")
    w2_d = nc.dram_tensor("w2p", [G, 128, IT, NG], BF16, kind="ExternalInput")
    out_d = nc.dram_tensor("out", [T, H], FP32, kind="ExternalOutput")

    with tile.TileContext(nc) as tc:
        with (
            tc.tile_pool(name="xt", bufs=2) as xt_pool,
            tc.tile_pool(name="w13", bufs=3) as w13_pool,
            tc.tile_pool(name="w2", bufs=2) as w2_pool,
            tc.tile_pool(name="h", bufs=2 * IT) as h_pool,
            tc.tile_pool(name="act", bufs=4) as act_pool,
            tc.tile_pool(name="ob", bufs=4) as ob_pool,
            tc.tile_pool(name="pg", bufs=2, space="PSUM") as pg_pool,
            tc.tile_pool(name="pu", bufs=2, space="PSUM") as pu_pool,
            tc.tile_pool(name="po", bufs=2, space="PSUM") as po_pool,
        ):
            XSPLIT = 4 if HK % 4 == 0 else 1
            for b in range(NB):
                t0 = b * TB
                # xt on the gpsimd SWDGE queue, split into XSPLIT pieces:
                # decouples from the weight-panel HWDGE ring so the next
                # block's activations land before the PE drains this block.
                xt_sb = xt_pool.tile([128, HK, TB], BF16)
                kstep = HK // XSPLIT
                for xs in range(XSPLIT):
                    k0 = xs * kstep
                    nc.gpsimd.dma_start(
                        xt_sb[:, k0 : k0 + kstep, :],
                        xt_d[:, k0 : k0 + kstep, t0 : t0 + TB],
                    )
                h_tiles = []
                for i in range(IT):
                    w1_sb = w13_pool.tile([128, HK, 128], BF16, tag="w1")
                    nc.sync.dma_start(w1_sb[:], w1_d[i])
                    w3_sb = w13_pool.tile([128, HK, 128], BF16, tag="w3")
                    nc.sync.dma_start(w3_sb[:], w3_d[i])
                    pg = pg_pool.tile([128, TB], FP32)
                    pu = pu_pool.tile([128, TB], FP32)
                    for k in range(HK):
                        nc.tensor.matmul(
                            pg[:], w1_sb[:, k, :], xt_sb[:, k, :],
                            start=(k == 0), stop=(k == HK - 1),
                        )
                    for k in range(HK):
                        nc.tensor.matmul(
                            pu[:], w3_sb[:, k, :], xt_sb[:, k, :],
                            start=(k == 0), stop=(k == HK - 1),
                        )
                    sig = act_pool.tile([128, TB], FP32, tag="sig")
                    nc.scalar.activation(
                        sig[:], pg[:], mybir.ActivationFunctionType.Sigmoid
                    )
                    t1 = act_pool.tile([128, TB], FP32, tag="t1")
                    nc.vector.tensor_mul(t1[:], sig[:], pg[:])
                    h = h_pool.tile([128, TB], BF16)
                    nc.vector.tensor_mul(h[:], t1[:], pu[:])
                    h_tiles.append(h)
                for g in range(G):
                    w2_sb = w2_pool.tile([128, IT, NG], BF16)
                    nc.scalar.dma_start(w2_sb[:], w2_d[g])
                    for s in range(TS):
                        po = po_pool.tile([128, NG], FP32)
                        for c in range(IT):
                            nc.tensor.matmul(
                                po[:],
                                h_tiles[c][:, s * 128 : (s + 1) * 128],
                                w2_sb[:, c, :],
                                start=(c == 0), stop=(c == IT - 1),
                            )
                        ob = ob_pool.tile([128, NG], FP32)
                        nc.vector.tensor_copy(ob[:], po[:])
                        nc.scalar.dma_start(
                            out_d[t0 + s * 128 : t0 + (s + 1) * 128,
                                  g * NG : (g + 1) * NG],
                            ob[:],
                        )
    nc.compile()
    return nc


def prep_inputs(x, w1, w2, w3, ncores, TB, NG):
    """Host-side shard + layout prep. Returns per-core input maps."""
    T, H = x.shape
    I = w1.shape[0]
    iloc = I // ncores
    HK = H // 128
    IT = iloc // 128
    G = H // NG

    xt = np.ascontiguousarray(
        x.reshape(T, HK, 128).transpose(2, 1, 0)
    ).astype(NP_BF16)

    in_maps = []
    for c in range(ncores):
        i0 = c * iloc
        w1p = (
            w1[i0 : i0 + iloc]
            .reshape(IT, 128, HK, 128)
            .transpose(0, 3, 2, 1)
            .astype(NP_BF16)
        )
        w3p = (
            w3[i0 : i0 + iloc]
            .reshape(IT, 128, HK, 128)
            .transpose(0, 3, 2, 1)
            .astype(NP_BF16)
        )
        w2p = (
            w2[:, i0 : i0 + iloc]
            .reshape(G, NG, IT, 128)
            .transpose(0, 3, 2, 1)
            .astype(NP_BF16)
        )
        in_maps.append(
            {
                "xt": xt,
                "w1p": np.ascontiguousarray(w1p),
                "w3p": np.ascontiguousarray(w3p),
                "w2p": np.ascontiguousarray(w2p),
            }
        )
    return in_maps


_NC_CACHE = {}


def _get_nc(T, H, iloc, TB, NG):
    key = (T, H, iloc, TB, NG)
    if key not in _NC_CACHE:
        _NC_CACHE[key] = build_ffn_bass(T, H, iloc, TB, NG)
    return _NC_CACHE[key]


def run(x, w1, w2, w3, ncores=NCORES, TB=512, NG=512, **spmd_kwargs):
    """Full pipeline; returns (output, BassKernelResults)."""
    T, H = x.shape
    iloc = w1.shape[0] // ncores
    nc = _get_nc(T, H, iloc, TB, NG)
    in_maps = prep_inputs(x, w1, w2, w3, ncores, TB, NG)
    res = bass_utils.run_bass_kernel_spmd(
        nc, in_maps, list(range(ncores)), **spmd_kwargs
    )
    out = res.results[0]["out"]
    for c in range(1, ncores):
        out = out + res.results[c]["out"]
    return np.ascontiguousarray(out, dtype=np.float32), res


def kernel(x, w1, w2, w3):
    out, _ = run(
        np.asarray(x, dtype=np.float32),
        np.asarray(w1, dtype=np.float32),
        np.asarray(w2, dtype=np.float32),
        np.asarray(w3, dtype=np.float32),
    )
    return out


# revision 17
# speedup vs baseline: 1.0102x; 1.0079x over previous
"""SwiGLU FFN (dense MoE expert) on 8 TRN2 NeuronCores, tensor-parallel.

Reference computation (fp32):
    out = (silu(x @ w1.T) * (x @ w3.T)) @ w2.T
    x  [T=4096, H=4096]
    w1 [I=14336, H]  gate proj
    w3 [I=14336, H]  up proj
    w2 [H, I]        down proj

Sharding: tensor-parallel over the intermediate dim I. Each of the 8
cores owns I/8 = 1792 rows of w1/w3 and the matching 1792 columns of
w2, computes a full-shape partial output, and the host sums the 8
partials (the "all-reduce" of the row-parallel down projection).

On-core dataflow (per token block TB):
    A:  gate^T[i,t] = sum_h w1[i,h] x[t,h]   (psum, bf16 matmuls)
    B:  up^T  [i,t] = sum_h w3[i,h] x[t,h]
        h^T = silu(gate^T) * up^T            (ACT + DVE, -> bf16)
    C:  out[t,hd]  += sum_i h^T[i,t] w2[hd,i]

All matmul operands are pre-laid-out on the host so every DMA is a
contiguous [128, big] block and every matmul is a full 128-contraction
with a 512-wide moving operand.
"""

import numpy as np
import ml_dtypes

import concourse.bass as bass
import concourse.mybir as mybir
import concourse.tile as tile
from concourse import bacc, bass_utils

BF16 = mybir.dt.bfloat16
FP32 = mybir.dt.float32
NP_BF16 = ml_dtypes.bfloat16

TOKENS = 4096
HIDDEN = 4096
INTER = 14336
NCORES = 8
ILOC = INTER // NCORES  # 1792 intermediate rows per core


def build_ffn_bass(T, H, iloc, TB, NG):
    """Per-core Bass program (identical on all cores; SPMD over shards).

    T tokens, H hidden, iloc local intermediate rows, TB token block
    (<=512, psum free dim), NG down-proj output group width (<=512).

    DRAM layouts (host-prepped, see kernel()):
      xt  [128, HK, T]        bf16  xt[p,k,t]   = x[t, 128k+p]
      w1p [IT, 128, HK, 128]  bf16  w1p[i,p,k,j]= w1loc[128i+j, 128k+p]
      w3p same as w1p
      w2p [G, 128, IT, NG]    bf16  w2p[g,p,c,j]= w2[g*NG+j, i0+128c+p]
      out [T, H] fp32 partial (summed across cores on the host)
    """
    HK = H // 128
    IT = iloc // 128
    NB = T // TB
    G = H // NG
    TS = TB // 128
    assert TB <= 512 and NG <= 512

    nc = bacc.Bacc("TRN2", num_devices=NCORES)
    xt_d = nc.dram_tensor("xt", [128, HK, T], BF16, kind="ExternalInput")
    w1_d = nc.dram_tensor("w1p", [IT, 128, HK, 128], BF16, kind="ExternalInput")
    w3_d = nc.dram_tensor("w3p", [IT, 128, HK, 128], BF16, kind="ExternalInput")
    w2_d = nc.dram_tensor("w2p", [G, 128, IT, NG], BF16, kind="ExternalInput")
    out_d = nc.dram_tensor("out", [T, H], FP32, kind="ExternalOutput")

    XSPLIT = 4 if HK % 4 == 0 else 1
    with tile.TileContext(nc) as tc:
        with (
            tc.tile_pool(name="xt", bufs=2 * XSPLIT) as xt_pool,
            tc.tile_pool(name="w13", bufs=3) as w13_pool,
            tc.tile_pool(name="w2", bufs=2) as w2_pool,
            tc.tile_pool(name="h", bufs=2 * IT) as h_pool,
            tc.tile_pool(name="act", bufs=4) as act_pool,
            tc.tile_pool(name="ob", bufs=4) as ob_pool,
            tc.tile_pool(name="pg", bufs=2, space="PSUM") as pg_pool,
            tc.tile_pool(name="pu", bufs=2, space="PSUM") as pu_pool,
            tc.tile_pool(name="po", bufs=2, space="PSUM") as po_pool,
        ):
            for b in range(NB):
                t0 = b * TB
                # xt on the gpsimd SWDGE queue, as XSPLIT independent tiles:
                # decouples from the weight-panel HWDGE ring, and the first
                # matmuls only wait on the first sub-tile.
                kstep = HK // XSPLIT
                xt_subs = []
                for xs in range(XSPLIT):
                    k0 = xs * kstep
                    xt_sb = xt_pool.tile([128, kstep, TB], BF16)
                    nc.gpsimd.dma_start(
                        xt_sb[:], xt_d[:, k0 : k0 + kstep, t0 : t0 + TB]
                    )
                    xt_subs.append(xt_sb)
                h_tiles = []
                for i in range(IT):
                    w1_sb = w13_pool.tile([128, HK, 128], BF16, tag="w1")
                    nc.sync.dma_start(w1_sb[:], w1_d[i])
                    w3_sb = w13_pool.tile([128, HK, 128], BF16, tag="w3")
                    nc.sync.dma_start(w3_sb[:], w3_d[i])
                    pg = pg_pool.tile([128, TB], FP32)
                    pu = pu_pool.tile([128, TB], FP32)
                    for k in range(HK):
                        nc.tensor.matmul(
                            pg[:], w1_sb[:, k, :],
                            xt_subs[k // kstep][:, k % kstep, :],
                            start=(k == 0), stop=(k == HK - 1),
                        )
                    for k in range(HK):
                        nc.tensor.matmul(
                            pu[:], w3_sb[:, k, :],
                            xt_subs[k // kstep][:, k % kstep, :],
                            start=(k == 0), stop=(k == HK - 1),
                        )
                    sig = act_pool.tile([128, TB], FP32, tag="sig")
                    nc.scalar.activation(
                        sig[:], pg[:], mybir.ActivationFunctionType.Sigmoid
                    )
                    t1 = act_pool.tile([128, TB], FP32, tag="t1")
                    nc.vector.tensor_mul(t1[:], sig[:], pg[:])
                    h = h_pool.tile([128, TB], BF16)
                    nc.vector.tensor_mul(h[:], t1[:], pu[:])
                    h_tiles.append(h)
                for g in range(G):
                    w2_sb = w2_pool.tile([128, IT, NG], BF16)
                    nc.sync.dma_start(w2_sb[:], w2_d[g])
                    for s in range(TS):
                        po = po_pool.tile([128, NG], FP32)
                        for c in range(IT):
                            nc.tensor.matmul(
                                po[:],
                                h_tiles[c][:, s * 128 : (s + 1) * 128],
                                w2_sb[:, c, :],
                                start=(c == 0), stop=(c == IT - 1),
                            )
                        ob = ob_pool.tile([128, NG], FP32)
                        nc.vector.tensor_copy(ob[:], po[:])
                        nc.scalar.dma_start(
                            out_d[t0 + s * 128 : t0 + (s + 1) * 128,
                                  g * NG : (g + 1) * NG],
                            ob[:],
                        )
    nc.compile()
    return nc


def prep_inputs(x, w1, w2, w3, ncores, TB, NG):
    """Host-side shard + layout prep. Returns per-core input maps."""
    T, H = x.shape
    I = w1.shape[0]
    iloc = I // ncores
    HK = H // 128
    IT = iloc // 128
    G = H // NG

    xt = np.ascontiguousarray(
        x.reshape(T, HK, 128).transpose(2, 1, 0)
    ).astype(NP_BF16)

    in_maps = []
    for c in range(ncores):
        i0 = c * iloc
        w1p = (
            w1[i0 : i0 + iloc]
            .reshape(IT, 128, HK, 128)
            .transpose(0, 3, 2, 1)
            .astype(NP_BF16)
        )
        w3p = (
            w3[i0 : i0 + iloc]
            .reshape(IT, 128, HK, 128)
            .transpose(0, 3, 2, 1)
            .astype(NP_BF16)
        )
        w2p = (
            w2[:, i0 : i0 + iloc]
            .reshape(G, NG, IT, 128)
            .transpose(0, 3, 2, 1)
            .astype(NP_BF16)
        )
        in_maps.append(
            {
                "xt": xt,
                "w1p": np.ascontiguousarray(w1p),
                "w3p": np.ascontiguousarray(w3p),
                "w2p": np.ascontiguousarray(w2p),
            }
        )
    return in_maps


_NC_CACHE = {}


def _get_nc(T, H, iloc, TB, NG):
    key = (T, H, iloc, TB, NG)
    if key not in _NC_CACHE:
        _NC_CACHE[key] = build_ffn_bass(T, H, iloc, TB, NG)
    return _NC_CACHE[key]


def run(x, w1, w2, w3, ncores=NCORES, TB=512, NG=512, **spmd_kwargs):
    """Full pipeline; returns (output, BassKernelResults)."""
    T, H = x.shape
    iloc = w1.shape[0] // ncores
    nc = _get_nc(T, H, iloc, TB, NG)
    in_maps = prep_inputs(x, w1, w2, w3, ncores, TB, NG)
    res = bass_utils.run_bass_kernel_spmd(
        nc, in_maps, list(range(ncores)), **spmd_kwargs
    )
    out = res.results[0]["out"]
    for c in range(1, ncores):
        out = out + res.results[c]["out"]
    return np.ascontiguousarray(out, dtype=np.float32), res


def kernel(x, w1, w2, w3):
    out, _ = run(
        np.asarray(x, dtype=np.float32),
        np.asarray(w1, dtype=np.float32),
        np.asarray(w2, dtype=np.float32),
        np.asarray(w3, dtype=np.float32),
    )
    return out
